# revision 28
# baseline (speedup 1.0000x reference)
"""Trainium2 Bass kernel for nn_MetaNetLinearizedModel (v14: no-collective
F-sharding + fp8 DoubleRow U-stream, pair-major).

Each core owns a 96-column slice fc of the feature dim F=768 and computes,
fully locally (no AllReduce):
    z1_c  = X @ W1[:, fc]                      (bf16, f32 accum)
    g_c   = gelu(z1_c + b1[fc]) ;  gp_c = gelu'(...) via central difference
    U_t,c = X @ dW1[t][:, fc]                  (fp8 x fp8 DoubleRow)
    v_t,c = gp_c * (U_t,c + db1[t][fc])
    P_t,c = v_t,c @ W2[fc, :] + g_c @ dW2[t][fc, :]    -> PO rows 16t:16t+16
    fo_c  = g_c @ W2[fc, :]                            -> FO (feats partial)
The host sums partials across cores, runs the tiny meta-net for coefs, and
forms  out = feats + b2 + sum_t coefs[:,t] * P_t + coefs @ db2.

The dW1 stream is PAIR-major: each ring streams its two task-pairs
sequentially (sync: (t0,t1),(t2,t3); scalar: (t4,t5),(t6,t7)), so a pair's
U finishes mid-stream and its reduce/v-term/PO-export drain overlaps the
remaining stream; only the final pair's short chain trails the last DMA.
tile_wait_until pins (all at/below real DMA-arrival times, so they never
add waiting) force the tile scheduler's static per-engine order to match
the real arrival order; the last two scalar-ring dma_starts are pinned
past the gelu activations' sim time so the activations are not queued
behind their DMA-semaphore rotation waits.
"""
import sys

sys.path.insert(0, "/opt/trn_rl_repo")

import numpy as np
import ml_dtypes
import concourse.bass as bass
import concourse.bacc as bacc
import concourse.tile as tile
import concourse.mybir as mybir
from concourse import bass_utils

F32 = mybir.dt.float32
BF16 = mybir.dt.bfloat16
FP8 = mybir.dt.float8e4
AF = mybir.ActivationFunctionType
OP = mybir.AluOpType
PM = mybir.MatmulPerfMode

B = 16
D = 3 * 64 * 64        # 12288
F = 768
HID = 192
T = 8
NCORES = 8
FSH = F // NCORES      # 96 columns of F per core
KD = D // 128          # 96 k-tiles
FP8_SCALE = 32.0       # dW1 fp8 scale
FP8_XS = 4.0           # X fp8 scale (U stream stationary)
EPS = 0.125            # central-difference step for gelu'
GPD_S = 4.0 / (FP8_SCALE * FP8_XS)

PW = 2 * FSH           # 192 dW1 columns per k-tile per task pair
W1KA = 48              # w1 slice k-tiles on the scalar ring
# per-pair chunk boundaries in k-tiles (even, for DoubleRow); the tiny
# first chunk lets U matmuls start ~1.5us after the preamble
CHUNKS = [(0, 0, 8), (0, 8, 56), (0, 56, 96), (1, 0, 48), (1, 48, 96)]

_CACHE = {}


def build():
    nc = bacc.Bacc("TRN2", target_bir_lowering=False, debug=False,
                   enable_asserts=False, num_devices=NCORES)

    XT = nc.dram_tensor("xt", [128, KD * B], BF16, kind="ExternalInput")
    XTF8 = nc.dram_tensor("xtf8", [128, KD * B], FP8, kind="ExternalInput")
    W1S = nc.dram_tensor("w1s", [128, KD * FSH], BF16, kind="ExternalInput")
    DW1A = nc.dram_tensor("dw1a", [128, 2 * KD * PW], FP8,
                          kind="ExternalInput")
    DW1B = nc.dram_tensor("dw1b", [128, 2 * KD * PW], FP8,
                          kind="ExternalInput")
    W2P = nc.dram_tensor("w2p", [FSH, F], BF16, kind="ExternalInput")
    DW2P = nc.dram_tensor("dw2p", [FSH, T * F], BF16, kind="ExternalInput")
    CONS = nc.dram_tensor("cons", [128, 35], F32, kind="ExternalInput")
    DB1R = nc.dram_tensor("db1r", [B, F], F32, kind="ExternalInput")
    PO = nc.dram_tensor("po", [128, F], BF16, kind="ExternalOutput")
    FO = nc.dram_tensor("fo", [B, F], F32, kind="ExternalOutput")

    with tile.TileContext(nc, num_cores=NCORES) as tc:
        with (
            tc.tile_pool(name="cst", bufs=1) as cst,
            tc.tile_pool(name="dwc", bufs=1) as dwc,
            tc.tile_pool(name="wrk", bufs=1) as wrk,
            tc.tile_pool(name="psq", bufs=1, space="PSUM") as psq,
            tc.tile_pool(name="psu", bufs=1, space="PSUM") as psu,
            tc.tile_pool(name="pss", bufs=2, space="PSUM") as pss,
        ):
            # ---- activation LUT preload ----
            scr = wrk.tile([1, 2], F32)
            nc.vector.memset(scr[:], 0.0)
            scr2 = wrk.tile([1, 2], F32)
            nc.scalar.activation(scr2[:, 0:1], scr[:, 0:1], AF.Gelu_apprx_tanh)

            # ---- DMA kicks ----
            # both rings lead with tiny PE-feeding transfers so U matmuls
            # start ~1.5us in; z1's w1s halves ride mid-ring; dW2 rides
            # mid-sync just before the g-terms need it
            # sync:   xtf8, dwa c0(8k), dwa c1, w1s[48:96], dw2p, dwa c2-c4
            # scalar: xt, dwb c0(8k), w1s[0:48], w2p, dwb c1-c4
            # gpsimd: consts in, fo out
            def mk_chunk(dram, which, i):
                q, k0, k1 = CHUNKS[i]
                t_ = dwc.tile([128, (k1 - k0) * PW], FP8, name="dwt",
                              tag=f"dw{which}{i}")
                src = dram.ap()[:, (q * KD + k0) * PW:(q * KD + k1) * PW]
                return t_, src

            xtf8_sb = cst.tile([128, KD * B], FP8)
            nc.sync.dma_start(xtf8_sb[:], XTF8.ap())
            dwa = [None] * len(CHUNKS)
            dwa[0], src = mk_chunk(DW1A, "a", 0)
            nc.sync.dma_start(dwa[0][:], src)
            dwa[1], src = mk_chunk(DW1A, "a", 1)
            nc.sync.dma_start(dwa[1][:], src)
            w1s_sb = cst.tile([128, KD * FSH], BF16)
            nc.sync.dma_start(w1s_sb[:, W1KA * FSH:KD * FSH],
                              W1S.ap()[:, W1KA * FSH:KD * FSH])
            dw2_sb = cst.tile([FSH, T * F], BF16)
            nc.sync.dma_start(dw2_sb[:], DW2P.ap())
            for i in range(2, len(CHUNKS)):
                dwa[i], src = mk_chunk(DW1A, "a", i)
                nc.sync.dma_start(dwa[i][:], src)

            xt_sb = cst.tile([128, KD * B], BF16)
            nc.scalar.dma_start(xt_sb[:], XT.ap())
            dwb = [None] * len(CHUNKS)
            dwb[0], src = mk_chunk(DW1B, "b", 0)
            nc.scalar.dma_start(dwb[0][:], src)
            nc.scalar.dma_start(w1s_sb[:, 0:W1KA * FSH],
                                W1S.ap()[:, 0:W1KA * FSH])
            w2_sb = cst.tile([FSH, F], BF16)
            nc.scalar.dma_start(w2_sb[:], W2P.ap())
            # the last two scalar dma_starts are pinned past the gelu
            # activations' sim time so the activations are not stuck
            # behind their semaphore-rotation waits on the scalar engine
            latepins = {3: 0.028, 4: 0.030}
            for i in range(1, len(CHUNKS)):
                dwb[i], src = mk_chunk(DW1B, "b", i)
                with tc.tile_wait_until(latepins.get(i, 0),
                                        enable=(i in latepins)):
                    nc.scalar.dma_start(dwb[i][:], src)
            cons_sb = cst.tile([128, 35], F32)
            nc.gpsimd.dma_start(cons_sb[:], CONS.ap())
            db1r_sb = cst.tile([B, F], F32)
            nc.gpsimd.dma_start(db1r_sb[:], DB1R.ap())

            eye = cons_sb[0:32, 0:32]
            b1c = cons_sb[0:FSH, 32:33]
            b1p = cons_sb[0:FSH, 33:34]
            b1m = cons_sb[0:FSH, 34:35]
            xtf8_3 = xtf8_sb[:].rearrange("p (k b) -> p k b", b=B)

            # ---- z1 = X @ W1[:, fc]; two k-parity chains in separate
            # PSUM banks so consecutive matmuls pipeline (same-bank
            # accumulation halves the PE matmul rate) ----
            z1psE = pss.tile([B, FSH], F32, name="sp", tag="sp",
                             padded_shape=[128, 512])
            z1psO = pss.tile([B, FSH], F32, name="sp", tag="sp",
                             padded_shape=[128, 512])
            # z1 pinned near its real w1s arrival so its matmuls do not
            # head-of-line block the (earlier-fed) U stream on the PE
            for i, k in enumerate(range(KD)):
                zp = z1psE if i % 2 == 0 else z1psO
                with tc.tile_wait_until(0.016 if k < W1KA else 0.023):
                    nc.tensor.matmul(zp[:], xt_sb[:, k * B:(k + 1) * B],
                                     w1s_sb[:, k * FSH:(k + 1) * FSH],
                                     start=(i < 2), stop=(i >= KD - 2),
                                     skip_group_check=True)

            z1pad0 = wrk.tile([32, FSH], F32)
            nc.vector.tensor_copy(z1pad0[0:B, :], z1psE[:])
            z1pad = wrk.tile([32, FSH], F32)
            nc.vector.tensor_add(z1pad[0:B, :], z1psO[:], z1pad0[0:B, :])
            z1tp = pss.tile([FSH, 32], F32, name="sp", tag="sp",
                            padded_shape=[128, 512])
            nc.tensor.matmul(z1tp[:], z1pad[:], eye, is_transpose=True,
                             skip_group_check=True)
            z1t = wrk.tile([FSH, B], F32)
            nc.vector.tensor_copy(z1t[:], z1tp[:, 0:B])

            gT = wrk.tile([FSH, B], BF16)
            nc.scalar.activation(gT[:], z1t[:], AF.Gelu_apprx_tanh, bias=b1c)
            gpp = wrk.tile([FSH, B], F32)
            nc.scalar.activation(gpp[:], z1t[:], AF.Gelu_apprx_tanh, bias=b1p)
            gpm = wrk.tile([FSH, B], F32)
            nc.scalar.activation(gpm[:], z1t[:], AF.Gelu_apprx_tanh, bias=b1m)
            gpdr = wrk.tile([FSH, B], F32)
            nc.vector.tensor_sub(gpdr[:], gpp[:], gpm[:])
            gpd = wrk.tile([FSH, B], F32)
            nc.vector.tensor_scalar_mul(gpd[:], gpdr[:], GPD_S)

            gpe = wrk.tile([FSH, 32], BF16)
            nc.vector.memset(gpe[:], 0.0)
            gpo = wrk.tile([FSH, 32], BF16)
            nc.vector.memset(gpo[:], 0.0)
            nc.vector.tensor_copy(gpe[:, 0:16], gT[:])
            nc.vector.tensor_copy(gpo[:, 16:32], gT[:])

            # ---- feats partial ----
            fps5 = pss.tile([B, 512], F32, name="sp", tag="sp",
                            padded_shape=[128, 512])
            nc.tensor.matmul(fps5[:], gT[:], w2_sb[:, 0:512],
                             start=True, stop=True, skip_group_check=True)
            fps2 = pss.tile([B, 256], F32, name="sp", tag="sp",
                            padded_shape=[128, 512])
            nc.tensor.matmul(fps2[:], gT[:], w2_sb[:, 512:F],
                             start=True, stop=True, skip_group_check=True)
            fo_sb = wrk.tile([B, F], F32)
            nc.vector.tensor_copy(fo_sb[:, 0:512], fps5[:])
            nc.vector.tensor_copy(fo_sb[:, 512:F], fps2[:])
            nc.gpsimd.dma_start(FO.ap(), fo_sb[:])

            # ---- P accumulation: pair group gq owns rows 32gq:32gq+32 ----
            P5 = psu.tile([128, 512], F32, name="p5")
            P2 = psu.tile([128, 256], F32, name="p2",
                          padded_shape=[128, 512])
            vps = wrk.tile([FSH, 128], BF16)
            po_sb = wrk.tile([128, F], BF16)

            def mm_gterm(t):
                j = t // 2
                st = gpe if t % 2 == 0 else gpo
                mv = dw2_sb[:, t * F:t * F + 512]
                nc.tensor.matmul(P5[32 * j:32 * j + 32, :], st[:], mv,
                                 start=(t % 2 == 0), stop=False,
                                 tile_position=(0, 32 * j),
                                 skip_group_check=True)
                mv = dw2_sb[:, t * F + 512:(t + 1) * F]
                nc.tensor.matmul(P2[32 * j:32 * j + 32, :], st[:], mv,
                                 start=(t % 2 == 0), stop=False,
                                 tile_position=(0, 32 * j),
                                 skip_group_check=True)

            for t in range(T):
                mm_gterm(t)

            # ---- U accumulators: each pair uses two k-parity chains in
            # SEPARATE PSUM banks (same-bank back-to-back accumulation
            # halves the PE rate); 8 chain tiles rotate through 4 banks,
            # so a pair's banks are recycled after its drain reads them.
            # Creation order matches streaming order (A0, B0, A1, B1).
            upt = {}
            for gq in (0, 2, 1, 3):   # stream order: A-pair0, B-pair0, ...
                for cg in range(2):
                    upt[(gq, cg)] = psq.tile([B, PW], F32,
                                             name=f"u{gq}{cg}", tag="u",
                                             bufs=4,
                                             padded_shape=[128, 512])

            def chunk_u(gq, t_, k0, k1):
                ch3 = t_[:].rearrange("p (k f) -> p k f", f=PW)
                for lp in range((k1 - k0) // 2):
                    kp = k0 // 2 + lp
                    cg = kp % 2
                    nc.tensor.matmul(upt[(gq, cg)][:],
                                     xtf8_3[:, 2 * kp:2 * kp + 2, :],
                                     ch3[:, 2 * lp:2 * lp + 2, :],
                                     start=(kp < 2),
                                     stop=(kp >= KD // 2 - 2),
                                     perf_mode=PM.DoubleRow,
                                     skip_group_check=True)

            def drain_pair(gq):
                for tt in range(2):
                    t = 2 * gq + tt
                    tsum = wrk.tile([B, FSH], F32, name="ts", tag="ts",
                                    bufs=2)
                    nc.vector.tensor_add(
                        tsum[:],
                        upt[(gq, 0)][:, tt * FSH:(tt + 1) * FSH],
                        db1r_sb[:, t * FSH:(t + 1) * FSH])
                    tzpad = wrk.tile([32, FSH], F32, name="tz", tag="tz",
                                     bufs=2)
                    nc.vector.tensor_add(
                        tzpad[0:B, :],
                        upt[(gq, 1)][:, tt * FSH:(tt + 1) * FSH],
                        tsum[:])
                    tztp = pss.tile([FSH, 32], F32, name="sp", tag="sp",
                                    padded_shape=[128, 512])
                    nc.tensor.matmul(tztp[:], tzpad[:], eye,
                                     is_transpose=True,
                                     skip_group_check=True)
                    nc.vector.tensor_mul(vps[:, t * B:(t + 1) * B],
                                         tztp[:, 0:B], gpd[:])
                ro = 32 * gq
                nc.tensor.matmul(P5[ro:ro + 32, :], vps[:, ro:ro + 32],
                                 w2_sb[:, 0:512],
                                 start=False, stop=True,
                                 tile_position=(0, ro),
                                 skip_group_check=True)
                nc.tensor.matmul(P2[ro:ro + 32, :], vps[:, ro:ro + 32],
                                 w2_sb[:, 512:F],
                                 start=False, stop=True,
                                 tile_position=(0, ro),
                                 skip_group_check=True)
                nc.scalar.activation(po_sb[ro:ro + 32, 0:512],
                                     P5[ro:ro + 32, :], AF.Copy)
                nc.vector.tensor_copy(po_sb[ro:ro + 32, 512:F],
                                      P2[ro:ro + 32, :])
                eng = nc.sync if gq < 2 else nc.scalar
                eng.dma_start(PO.ap()[ro:ro + 32, :], po_sb[ro:ro + 32, :])

            # ---- pinned emission in real-arrival order ----
            def piece(ring, i):
                q, k0, k1 = CHUNKS[i]
                gq = q + (0 if ring == "a" else 2)
                t_ = (dwa if ring == "a" else dwb)[i]
                return gq, t_, k0, k1

            sched = [
                ("c", "a", 0, 0.0095), ("c", "b", 0, 0.0105),
                ("c", "a", 1, 0.011), ("c", "b", 1, 0.018),
                ("c", "b", 2, 0.027), ("d", "b", None, 0.0295),
                ("c", "a", 2, 0.030), ("d", "a", None, 0.034),
                ("c", "b", 3, 0.035), ("c", "a", 3, 0.036),
                ("c", "b", 4, 0.038), ("d", "b", None, 0.042),
                ("c", "a", 4, 0.043), ("d", "a", None, 0.046),
            ]
            drained = {"a": 0, "b": 0}
            for kind, ring, i, pin in sched:
                with tc.tile_wait_until(pin):
                    if kind == "c":
                        chunk_u(*piece(ring, i))
                    else:
                        gq = drained[ring] + (0 if ring == "a" else 2)
                        drain_pair(gq)
                        drained[ring] += 1

    nc.compile()
    return nc


def _get_nc():
    if "nc" not in _CACHE:
        _CACHE["nc"] = build()
    return _CACHE["nc"]


def _prep_in_maps(x, W1, b1, W2, b2, mW1, mb1, mW2, mb2, dW1, db1, dW2, db2):
    f32 = np.float32
    bf16 = ml_dtypes.bfloat16
    fp8 = ml_dtypes.float8_e4m3
    X = np.ascontiguousarray(np.asarray(x, f32).reshape(B, D))
    XT = np.ascontiguousarray(X.T)
    XTl = XT.reshape(KD, 128, B).transpose(1, 0, 2).reshape(128, KD * B)
    XTb = np.ascontiguousarray(XTl).astype(bf16)
    XTf8 = np.ascontiguousarray(XTl * FP8_XS).astype(fp8)
    W1 = np.asarray(W1, f32)
    W2 = np.asarray(W2, f32)
    b1 = np.asarray(b1, f32)
    dW1 = np.asarray(dW1, f32)
    db1 = np.asarray(db1, f32)
    dW2 = np.asarray(dW2, f32)

    def pack_pairs(dw_half, fc):
        blocks = []
        for q in range(2):
            arr = (dw_half[2 * q:2 * q + 2, :, fc] * FP8_SCALE)
            arr = arr.transpose(1, 0, 2).reshape(KD, 128, PW)
            blocks.append(arr.transpose(1, 0, 2).reshape(128, KD * PW))
        return np.ascontiguousarray(np.concatenate(blocks, 1)).astype(fp8)

    in_maps = []
    for c in range(NCORES):
        fc = slice(c * FSH, (c + 1) * FSH)
        w1s = np.ascontiguousarray(
            W1[:, fc].reshape(KD, 128, FSH).transpose(1, 0, 2)
            .reshape(128, KD * FSH)).astype(bf16)
        w2p = np.ascontiguousarray(W2[fc, :]).astype(bf16)
        dw2p = np.ascontiguousarray(
            dW2[:, fc, :].transpose(1, 0, 2).reshape(FSH, T * F)).astype(bf16)
        cons = np.zeros((128, 35), f32)
        cons[0:32, 0:32] = np.eye(32, dtype=f32)
        cons[0:FSH, 32] = b1[fc]
        cons[0:FSH, 33] = b1[fc] + EPS
        cons[0:FSH, 34] = b1[fc] - EPS
        db1r = np.ascontiguousarray(np.broadcast_to(
            (FP8_SCALE * FP8_XS * db1[:, fc]).reshape(T * FSH), (B, F)))
        in_maps.append({
            "xt": XTb,
            "xtf8": XTf8,
            "w1s": w1s,
            "dw1a": pack_pairs(dW1[0:4], fc),
            "dw1b": pack_pairs(dW1[4:8], fc),
            "w2p": w2p,
            "dw2p": dw2p,
            "cons": cons,
            "db1r": db1r.astype(f32),
        })
    return in_maps


def run(inputs, trace=False, trace_cores=None, tmpdir=None):
    nc = _get_nc()
    in_maps = _prep_in_maps(**inputs)
    res = bass_utils.run_bass_kernel_spmd(
        nc, in_maps, core_ids=list(range(NCORES)), trace=trace,
        trace_cores=trace_cores, tmpdir=tmpdir)

    f64 = np.float64
    b2 = np.asarray(inputs["b2"], f64)
    mW1 = np.asarray(inputs["mW1"], f64)
    mb1 = np.asarray(inputs["mb1"], f64)
    mW2 = np.asarray(inputs["mW2"], f64)
    mb2 = np.asarray(inputs["mb2"], f64)
    db2 = np.asarray(inputs["db2"], f64)

    feats = np.zeros((B, F), f64)
    P = np.zeros((128, F), f64)
    for c in range(NCORES):
        feats += res.results[c]["fo"].astype(f64)
        P += res.results[c]["po"].astype(f64)
    feats += b2[None, :]
    h = np.maximum(feats @ mW1.T + mb1, 0.0)
    coefs = h @ mW2.T + mb2                     # [B, T]
    out = feats + coefs @ db2
    for t in range(T):
        out += coefs[:, t:t + 1] * P[t * B:(t + 1) * B]
    return out.astype(np.float32), res


def kernel(**inputs):
    out, _ = run(inputs, trace=False)
    return out


# revision 29
# speedup vs baseline: 1.0212x; 1.0212x over previous
"""Trainium2 Bass kernel for nn_MetaNetLinearizedModel (v14: no-collective
F-sharding + fp8 DoubleRow U-stream, pair-major).

Each core owns a 96-column slice fc of the feature dim F=768 and computes,
fully locally (no AllReduce):
    z1_c  = X @ W1[:, fc]                      (bf16, f32 accum)
    g_c   = gelu(z1_c + b1[fc]) ;  gp_c = gelu'(...) via central difference
    U_t,c = X @ dW1[t][:, fc]                  (fp8 x fp8 DoubleRow)
    v_t,c = gp_c * (U_t,c + db1[t][fc])
    P_t,c = v_t,c @ W2[fc, :] + g_c @ dW2[t][fc, :]    -> PO rows 16t:16t+16
    fo_c  = g_c @ W2[fc, :]                            -> FO (feats partial)
The host sums partials across cores, runs the tiny meta-net for coefs, and
forms  out = feats + b2 + sum_t coefs[:,t] * P_t + coefs @ db2.

The dW1 stream is PAIR-major: each ring streams its two task-pairs
sequentially (sync: (t0,t1),(t2,t3); scalar: (t4,t5),(t6,t7)), so a pair's
U finishes mid-stream and its reduce/v-term/PO-export drain overlaps the
remaining stream; only the final pair's short chain trails the last DMA.
tile_wait_until pins (all at/below real DMA-arrival times, so they never
add waiting) force the tile scheduler's static per-engine order to match
the real arrival order; the last two scalar-ring dma_starts are pinned
past the gelu activations' sim time so the activations are not queued
behind their DMA-semaphore rotation waits.
"""
import sys

sys.path.insert(0, "/opt/trn_rl_repo")

import numpy as np
import ml_dtypes
import concourse.bass as bass
import concourse.bacc as bacc
import concourse.tile as tile
import concourse.mybir as mybir
from concourse import bass_utils

F32 = mybir.dt.float32
BF16 = mybir.dt.bfloat16
FP8 = mybir.dt.float8e4
AF = mybir.ActivationFunctionType
OP = mybir.AluOpType
PM = mybir.MatmulPerfMode

B = 16
D = 3 * 64 * 64        # 12288
F = 768
HID = 192
T = 8
NCORES = 8
FSH = F // NCORES      # 96 columns of F per core
KD = D // 128          # 96 k-tiles
FP8_SCALE = 32.0       # dW1 fp8 scale
FP8_XS = 4.0           # X fp8 scale (U stream stationary)
EPS = 0.125            # central-difference step for gelu'
GPD_S = 4.0 / (FP8_SCALE * FP8_XS)

PW = 2 * FSH           # 192 dW1 columns per k-tile per task pair
W1KA = 48              # w1 slice k-tiles on the scalar ring
# per-pair chunk boundaries in k-tiles (even, for DoubleRow); the tiny
# first chunk lets U matmuls start ~1.5us after the preamble
CHUNKS = [(0, 0, 8), (0, 8, 56), (0, 56, 96), (1, 0, 48), (1, 48, 96)]

_CACHE = {}


def build():
    nc = bacc.Bacc("TRN2", target_bir_lowering=False, debug=False,
                   enable_asserts=False, num_devices=NCORES)

    XT = nc.dram_tensor("xt", [128, KD * B], BF16, kind="ExternalInput")
    XTF8 = nc.dram_tensor("xtf8", [128, KD * B], FP8, kind="ExternalInput")
    W1S = nc.dram_tensor("w1s", [128, KD * FSH], BF16, kind="ExternalInput")
    DW1A = nc.dram_tensor("dw1a", [128, 2 * KD * PW], FP8,
                          kind="ExternalInput")
    DW1B = nc.dram_tensor("dw1b", [128, 2 * KD * PW], FP8,
                          kind="ExternalInput")
    W2P = nc.dram_tensor("w2p", [FSH, F], BF16, kind="ExternalInput")
    DW2P = nc.dram_tensor("dw2p", [FSH, T * F], BF16, kind="ExternalInput")
    CONS = nc.dram_tensor("cons", [128, 35], F32, kind="ExternalInput")
    DB1R = nc.dram_tensor("db1r", [B, F], F32, kind="ExternalInput")
    PO = nc.dram_tensor("po", [128, F], BF16, kind="ExternalOutput")
    FO = nc.dram_tensor("fo", [B, F], F32, kind="ExternalOutput")

    with tile.TileContext(nc, num_cores=NCORES) as tc:
        with (
            tc.tile_pool(name="cst", bufs=1) as cst,
            tc.tile_pool(name="dwc", bufs=1) as dwc,
            tc.tile_pool(name="wrk", bufs=1) as wrk,
            tc.tile_pool(name="psq", bufs=1, space="PSUM") as psq,
            tc.tile_pool(name="psu", bufs=1, space="PSUM") as psu,
            tc.tile_pool(name="pss", bufs=2, space="PSUM") as pss,
        ):
            # ---- activation LUT preload ----
            scr = wrk.tile([1, 2], F32)
            nc.vector.memset(scr[:], 0.0)
            scr2 = wrk.tile([1, 2], F32)
            nc.scalar.activation(scr2[:, 0:1], scr[:, 0:1], AF.Gelu_apprx_tanh)

            # ---- DMA kicks ----
            # both rings lead with tiny PE-feeding transfers so U matmuls
            # start ~1.5us in; z1's w1s halves ride mid-ring; dW2 rides
            # mid-sync just before the g-terms need it
            # sync:   xtf8, dwa c0(8k), dwa c1, w1s[48:96], dw2p, dwa c2-c4
            # scalar: xt, dwb c0(8k), w1s[0:48], w2p, dwb c1-c4
            # gpsimd: consts in, fo out
            def mk_chunk(dram, which, i):
                q, k0, k1 = CHUNKS[i]
                t_ = dwc.tile([128, (k1 - k0) * PW], FP8, name="dwt",
                              tag=f"dw{which}{i}")
                src = dram.ap()[:, (q * KD + k0) * PW:(q * KD + k1) * PW]
                return t_, src

            xtf8_sb = cst.tile([128, KD * B], FP8)
            nc.sync.dma_start(xtf8_sb[:], XTF8.ap())
            dwa = [None] * len(CHUNKS)
            dwa[0], src = mk_chunk(DW1A, "a", 0)
            nc.sync.dma_start(dwa[0][:], src)
            dwa[1], src = mk_chunk(DW1A, "a", 1)
            nc.sync.dma_start(dwa[1][:], src)
            w1s_sb = cst.tile([128, KD * FSH], BF16)
            nc.sync.dma_start(w1s_sb[:, W1KA * FSH:KD * FSH],
                              W1S.ap()[:, W1KA * FSH:KD * FSH])
            dw2_sb = cst.tile([FSH, T * F], BF16)
            nc.sync.dma_start(dw2_sb[:], DW2P.ap())
            for i in range(2, len(CHUNKS)):
                dwa[i], src = mk_chunk(DW1A, "a", i)
                nc.sync.dma_start(dwa[i][:], src)

            xt_sb = cst.tile([128, KD * B], BF16)
            nc.scalar.dma_start(xt_sb[:], XT.ap())
            dwb = [None] * len(CHUNKS)
            dwb[0], src = mk_chunk(DW1B, "b", 0)
            nc.scalar.dma_start(dwb[0][:], src)
            nc.scalar.dma_start(w1s_sb[:, 0:W1KA * FSH],
                                W1S.ap()[:, 0:W1KA * FSH])
            w2_sb = cst.tile([FSH, F], BF16)
            nc.scalar.dma_start(w2_sb[:], W2P.ap())
            # the last two scalar dma_starts are pinned past the gelu
            # activations' sim time so the activations are not stuck
            # behind their semaphore-rotation waits on the scalar engine
            latepins = {3: 0.016, 4: 0.018}
            for i in range(1, len(CHUNKS)):
                dwb[i], src = mk_chunk(DW1B, "b", i)
                with tc.tile_wait_until(latepins.get(i, 0),
                                        enable=(i in latepins)):
                    nc.scalar.dma_start(dwb[i][:], src)
            cons_sb = cst.tile([128, 35], F32)
            nc.gpsimd.dma_start(cons_sb[:], CONS.ap())
            db1r_sb = cst.tile([B, F], F32)
            nc.gpsimd.dma_start(db1r_sb[:], DB1R.ap())

            eye = cons_sb[0:32, 0:32]
            b1c = cons_sb[0:FSH, 32:33]
            b1p = cons_sb[0:FSH, 33:34]
            b1m = cons_sb[0:FSH, 34:35]
            xtf8_3 = xtf8_sb[:].rearrange("p (k b) -> p k b", b=B)

            # ---- z1 = X @ W1[:, fc]; two k-parity chains in separate
            # PSUM banks so consecutive matmuls pipeline (same-bank
            # accumulation halves the PE matmul rate) ----
            z1psE = pss.tile([B, FSH], F32, name="sp", tag="sp",
                             padded_shape=[128, 512])
            z1psO = pss.tile([B, FSH], F32, name="sp", tag="sp",
                             padded_shape=[128, 512])
            for i, k in enumerate(range(KD)):
                zp = z1psE if i % 2 == 0 else z1psO
                nc.tensor.matmul(zp[:], xt_sb[:, k * B:(k + 1) * B],
                                 w1s_sb[:, k * FSH:(k + 1) * FSH],
                                 start=(i < 2), stop=(i >= KD - 2),
                                 skip_group_check=True)

            z1pad0 = wrk.tile([32, FSH], F32)
            nc.vector.tensor_copy(z1pad0[0:B, :], z1psE[:])
            z1pad = wrk.tile([32, FSH], F32)
            nc.vector.tensor_add(z1pad[0:B, :], z1psO[:], z1pad0[0:B, :])
            z1tp = pss.tile([FSH, 32], F32, name="sp", tag="sp",
                            padded_shape=[128, 512])
            nc.tensor.matmul(z1tp[:], z1pad[:], eye, is_transpose=True,
                             skip_group_check=True)
            z1t = wrk.tile([FSH, B], F32)
            nc.vector.tensor_copy(z1t[:], z1tp[:, 0:B])

            gT = wrk.tile([FSH, B], BF16)
            nc.scalar.activation(gT[:], z1t[:], AF.Gelu_apprx_tanh, bias=b1c)
            gpp = wrk.tile([FSH, B], F32)
            nc.scalar.activation(gpp[:], z1t[:], AF.Gelu_apprx_tanh, bias=b1p)
            gpm = wrk.tile([FSH, B], F32)
            nc.scalar.activation(gpm[:], z1t[:], AF.Gelu_apprx_tanh, bias=b1m)
            gpdr = wrk.tile([FSH, B], F32)
            nc.vector.tensor_sub(gpdr[:], gpp[:], gpm[:])
            gpd = wrk.tile([FSH, B], F32)
            nc.vector.tensor_scalar_mul(gpd[:], gpdr[:], GPD_S)

            gpe = wrk.tile([FSH, 32], BF16)
            nc.vector.memset(gpe[:], 0.0)
            gpo = wrk.tile([FSH, 32], BF16)
            nc.vector.memset(gpo[:], 0.0)
            nc.vector.tensor_copy(gpe[:, 0:16], gT[:])
            nc.vector.tensor_copy(gpo[:, 16:32], gT[:])

            # ---- feats partial ----
            fps5 = pss.tile([B, 512], F32, name="sp", tag="sp",
                            padded_shape=[128, 512])
            nc.tensor.matmul(fps5[:], gT[:], w2_sb[:, 0:512],
                             start=True, stop=True, skip_group_check=True)
            fps2 = pss.tile([B, 256], F32, name="sp", tag="sp",
                            padded_shape=[128, 512])
            nc.tensor.matmul(fps2[:], gT[:], w2_sb[:, 512:F],
                             start=True, stop=True, skip_group_check=True)
            fo_sb = wrk.tile([B, F], F32)
            nc.vector.tensor_copy(fo_sb[:, 0:512], fps5[:])
            nc.vector.tensor_copy(fo_sb[:, 512:F], fps2[:])
            nc.gpsimd.dma_start(FO.ap(), fo_sb[:])

            # ---- P accumulation: pair group gq owns rows 32gq:32gq+32 ----
            P5 = psu.tile([128, 512], F32, name="p5")
            P2 = psu.tile([128, 256], F32, name="p2",
                          padded_shape=[128, 512])
            vps = wrk.tile([FSH, 128], BF16)
            po_sb = wrk.tile([128, F], BF16)

            def mm_gterm(t):
                j = t // 2
                st = gpe if t % 2 == 0 else gpo
                mv = dw2_sb[:, t * F:t * F + 512]
                nc.tensor.matmul(P5[32 * j:32 * j + 32, :], st[:], mv,
                                 start=(t % 2 == 0), stop=False,
                                 tile_position=(0, 32 * j),
                                 skip_group_check=True)
                mv = dw2_sb[:, t * F + 512:(t + 1) * F]
                nc.tensor.matmul(P2[32 * j:32 * j + 32, :], st[:], mv,
                                 start=(t % 2 == 0), stop=False,
                                 tile_position=(0, 32 * j),
                                 skip_group_check=True)

            for t in range(T):
                mm_gterm(t)

            # ---- U accumulators: each pair uses two k-parity chains in
            # SEPARATE PSUM banks (same-bank back-to-back accumulation
            # halves the PE rate); 8 chain tiles rotate through 4 banks,
            # so a pair's banks are recycled after its drain reads them.
            # Creation order matches streaming order (A0, B0, A1, B1).
            upt = {}
            for gq in (0, 2, 1, 3):   # stream order: A-pair0, B-pair0, ...
                for cg in range(2):
                    upt[(gq, cg)] = psq.tile([B, PW], F32,
                                             name=f"u{gq}{cg}", tag="u",
                                             bufs=4,
                                             padded_shape=[128, 512])

            def chunk_u(gq, t_, k0, k1):
                ch3 = t_[:].rearrange("p (k f) -> p k f", f=PW)
                for lp in range((k1 - k0) // 2):
                    kp = k0 // 2 + lp
                    cg = kp % 2
                    nc.tensor.matmul(upt[(gq, cg)][:],
                                     xtf8_3[:, 2 * kp:2 * kp + 2, :],
                                     ch3[:, 2 * lp:2 * lp + 2, :],
                                     start=(kp < 2),
                                     stop=(kp >= KD // 2 - 2),
                                     perf_mode=PM.DoubleRow,
                                     skip_group_check=True)

            def drain_pair(gq):
                for tt in range(2):
                    t = 2 * gq + tt
                    tsum = wrk.tile([B, FSH], F32, name="ts", tag="ts",
                                    bufs=2)
                    nc.vector.tensor_add(
                        tsum[:],
                        upt[(gq, 0)][:, tt * FSH:(tt + 1) * FSH],
                        db1r_sb[:, t * FSH:(t + 1) * FSH])
                    tzpad = wrk.tile([32, FSH], F32, name="tz", tag="tz",
                                     bufs=2)
                    nc.vector.tensor_add(
                        tzpad[0:B, :],
                        upt[(gq, 1)][:, tt * FSH:(tt + 1) * FSH],
                        tsum[:])
                    tztp = pss.tile([FSH, 32], F32, name="sp", tag="sp",
                                    padded_shape=[128, 512])
                    nc.tensor.matmul(tztp[:], tzpad[:], eye,
                                     is_transpose=True,
                                     skip_group_check=True)
                    nc.vector.tensor_mul(vps[:, t * B:(t + 1) * B],
                                         tztp[:, 0:B], gpd[:])
                ro = 32 * gq
                nc.tensor.matmul(P5[ro:ro + 32, :], vps[:, ro:ro + 32],
                                 w2_sb[:, 0:512],
                                 start=False, stop=True,
                                 tile_position=(0, ro),
                                 skip_group_check=True)
                nc.tensor.matmul(P2[ro:ro + 32, :], vps[:, ro:ro + 32],
                                 w2_sb[:, 512:F],
                                 start=False, stop=True,
                                 tile_position=(0, ro),
                                 skip_group_check=True)
                nc.scalar.activation(po_sb[ro:ro + 32, 0:512],
                                     P5[ro:ro + 32, :], AF.Copy)
                nc.vector.tensor_copy(po_sb[ro:ro + 32, 512:F],
                                      P2[ro:ro + 32, :])
                eng = nc.sync if gq < 2 else nc.scalar
                eng.dma_start(PO.ap()[ro:ro + 32, :], po_sb[ro:ro + 32, :])

            # ---- pinned emission in real-arrival order ----
            def piece(ring, i):
                q, k0, k1 = CHUNKS[i]
                gq = q + (0 if ring == "a" else 2)
                t_ = (dwa if ring == "a" else dwb)[i]
                return gq, t_, k0, k1

            sched = [
                ("c", "a", 0, 0.0095), ("c", "b", 0, 0.0105),
                ("c", "a", 1, 0.011), ("c", "b", 1, 0.019),
                ("c", "b", 2, 0.024), ("d", "b", None, 0.026),
                ("c", "a", 2, 0.028), ("d", "a", None, 0.030),
                ("c", "b", 3, 0.031), ("c", "a", 3, 0.032),
                ("c", "b", 4, 0.034), ("d", "b", None, 0.038),
                ("c", "a", 4, 0.039), ("d", "a", None, 0.042),
            ]
            drained = {"a": 0, "b": 0}
            for kind, ring, i, pin in sched:
                with tc.tile_wait_until(pin):
                    if kind == "c":
                        chunk_u(*piece(ring, i))
                    else:
                        gq = drained[ring] + (0 if ring == "a" else 2)
                        drain_pair(gq)
                        drained[ring] += 1

    nc.compile()
    return nc


def _get_nc():
    if "nc" not in _CACHE:
        _CACHE["nc"] = build()
    return _CACHE["nc"]


def _prep_in_maps(x, W1, b1, W2, b2, mW1, mb1, mW2, mb2, dW1, db1, dW2, db2):
    f32 = np.float32
    bf16 = ml_dtypes.bfloat16
    fp8 = ml_dtypes.float8_e4m3
    X = np.ascontiguousarray(np.asarray(x, f32).reshape(B, D))
    XT = np.ascontiguousarray(X.T)
    XTl = XT.reshape(KD, 128, B).transpose(1, 0, 2).reshape(128, KD * B)
    XTb = np.ascontiguousarray(XTl).astype(bf16)
    XTf8 = np.ascontiguousarray(XTl * FP8_XS).astype(fp8)
    W1 = np.asarray(W1, f32)
    W2 = np.asarray(W2, f32)
    b1 = np.asarray(b1, f32)
    dW1 = np.asarray(dW1, f32)
    db1 = np.asarray(db1, f32)
    dW2 = np.asarray(dW2, f32)

    def pack_pairs(dw_half, fc):
        blocks = []
        for q in range(2):
            arr = (dw_half[2 * q:2 * q + 2, :, fc] * FP8_SCALE)
            arr = arr.transpose(1, 0, 2).reshape(KD, 128, PW)
            blocks.append(arr.transpose(1, 0, 2).reshape(128, KD * PW))
        return np.ascontiguousarray(np.concatenate(blocks, 1)).astype(fp8)

    in_maps = []
    for c in range(NCORES):
        fc = slice(c * FSH, (c + 1) * FSH)
        w1s = np.ascontiguousarray(
            W1[:, fc].reshape(KD, 128, FSH).transpose(1, 0, 2)
            .reshape(128, KD * FSH)).astype(bf16)
        w2p = np.ascontiguousarray(W2[fc, :]).astype(bf16)
        dw2p = np.ascontiguousarray(
            dW2[:, fc, :].transpose(1, 0, 2).reshape(FSH, T * F)).astype(bf16)
        cons = np.zeros((128, 35), f32)
        cons[0:32, 0:32] = np.eye(32, dtype=f32)
        cons[0:FSH, 32] = b1[fc]
        cons[0:FSH, 33] = b1[fc] + EPS
        cons[0:FSH, 34] = b1[fc] - EPS
        db1r = np.ascontiguousarray(np.broadcast_to(
            (FP8_SCALE * FP8_XS * db1[:, fc]).reshape(T * FSH), (B, F)))
        in_maps.append({
            "xt": XTb,
            "xtf8": XTf8,
            "w1s": w1s,
            "dw1a": pack_pairs(dW1[0:4], fc),
            "dw1b": pack_pairs(dW1[4:8], fc),
            "w2p": w2p,
            "dw2p": dw2p,
            "cons": cons,
            "db1r": db1r.astype(f32),
        })
    return in_maps


def run(inputs, trace=False, trace_cores=None, tmpdir=None):
    nc = _get_nc()
    in_maps = _prep_in_maps(**inputs)
    res = bass_utils.run_bass_kernel_spmd(
        nc, in_maps, core_ids=list(range(NCORES)), trace=trace,
        trace_cores=trace_cores, tmpdir=tmpdir)

    f64 = np.float64
    b2 = np.asarray(inputs["b2"], f64)
    mW1 = np.asarray(inputs["mW1"], f64)
    mb1 = np.asarray(inputs["mb1"], f64)
    mW2 = np.asarray(inputs["mW2"], f64)
    mb2 = np.asarray(inputs["mb2"], f64)
    db2 = np.asarray(inputs["db2"], f64)

    feats = np.zeros((B, F), f64)
    P = np.zeros((128, F), f64)
    for c in range(NCORES):
        feats += res.results[c]["fo"].astype(f64)
        P += res.results[c]["po"].astype(f64)
    feats += b2[None, :]
    h = np.maximum(feats @ mW1.T + mb1, 0.0)
    coefs = h @ mW2.T + mb2                     # [B, T]
    out = feats + coefs @ db2
    for t in range(T):
        out += coefs[:, t:t + 1] * P[t * B:(t + 1) * B]
    return out.astype(np.float32), res


def kernel(**inputs):
    out, _ = run(inputs, trace=False)
    return out


# revision 30
# speedup vs baseline: 1.1118x; 1.0887x over previous
"""Trainium2 Bass kernel for nn_MetaNetLinearizedModel (v14: no-collective
F-sharding + fp8 DoubleRow U-stream, pair-major).

Each core owns a 96-column slice fc of the feature dim F=768 and computes,
fully locally (no AllReduce):
    z1_c  = X @ W1[:, fc]                      (bf16, f32 accum)
    g_c   = gelu(z1_c + b1[fc]) ;  gp_c = gelu'(...) via central difference
    U_t,c = X @ dW1[t][:, fc]                  (fp8 x fp8 DoubleRow)
    v_t,c = gp_c * (U_t,c + db1[t][fc])
    P_t,c = v_t,c @ W2[fc, :] + g_c @ dW2[t][fc, :]    -> PO rows 16t:16t+16
    fo_c  = g_c @ W2[fc, :]                            -> FO (feats partial)
The host sums partials across cores, runs the tiny meta-net for coefs, and
forms  out = feats + b2 + sum_t coefs[:,t] * P_t + coefs @ db2.

The dW1 stream is PAIR-major: each ring streams its two task-pairs
sequentially (sync: (t0,t1),(t2,t3); scalar: (t4,t5),(t6,t7)), so a pair's
U finishes mid-stream and its reduce/v-term/PO-export drain overlaps the
remaining stream; only the final pair's short chain trails the last DMA.
tile_wait_until pins (all at/below real DMA-arrival times, so they never
add waiting) force the tile scheduler's static per-engine order to match
the real arrival order; the last two scalar-ring dma_starts are pinned
past the gelu activations' sim time so the activations are not queued
behind their DMA-semaphore rotation waits.
"""
import sys

sys.path.insert(0, "/opt/trn_rl_repo")

import numpy as np
import ml_dtypes
import concourse.bass as bass
import concourse.bacc as bacc
import concourse.tile as tile
import concourse.mybir as mybir
from concourse import bass_utils

F32 = mybir.dt.float32
BF16 = mybir.dt.bfloat16
FP8 = mybir.dt.float8e4
AF = mybir.ActivationFunctionType
OP = mybir.AluOpType
PM = mybir.MatmulPerfMode

B = 16
D = 3 * 64 * 64        # 12288
F = 768
HID = 192
T = 8
NCORES = 8
FSH = F // NCORES      # 96 columns of F per core
KD = D // 128          # 96 k-tiles
FP8_SCALE = 32.0       # dW1 fp8 scale
FP8_XS = 4.0           # X fp8 scale (U stream stationary)
EPS = 0.125            # central-difference step for gelu'
GPD_S = 4.0 / (FP8_SCALE * FP8_XS)

PW = 2 * FSH           # 192 dW1 columns per k-tile per task pair
W1KA = 48              # w1 slice k-tiles on the sync ring
# per-pair chunk boundaries in k-tiles (even, for DoubleRow)
CHUNKS = [(0, 0, 48), (0, 48, 96), (1, 0, 48), (1, 48, 88), (1, 88, 96)]

_CACHE = {}


def build():
    nc = bacc.Bacc("TRN2", target_bir_lowering=False, debug=False,
                   enable_asserts=False, num_devices=NCORES)

    XT = nc.dram_tensor("xt", [128, KD * B], BF16, kind="ExternalInput")
    XTF8 = nc.dram_tensor("xtf8", [128, KD * B], FP8, kind="ExternalInput")
    W1S = nc.dram_tensor("w1s", [128, KD * FSH], BF16, kind="ExternalInput")
    DW1A = nc.dram_tensor("dw1a", [128, 2 * KD * PW], FP8,
                          kind="ExternalInput")
    DW1B = nc.dram_tensor("dw1b", [128, 2 * KD * PW], FP8,
                          kind="ExternalInput")
    WD2 = nc.dram_tensor("wd2", [FSH, (T + 1) * F], BF16,
                         kind="ExternalInput")
    CONS = nc.dram_tensor("cons", [128, 35], F32, kind="ExternalInput")
    DB1R = nc.dram_tensor("db1r", [B, F], F32, kind="ExternalInput")
    PO = nc.dram_tensor("po", [128, F], BF16, kind="ExternalOutput")
    FO = nc.dram_tensor("fo", [B, F], F32, kind="ExternalOutput")

    with tile.TileContext(nc, num_cores=NCORES) as tc:
        with (
            tc.tile_pool(name="cst", bufs=1) as cst,
            tc.tile_pool(name="dwc", bufs=1) as dwc,
            tc.tile_pool(name="wrk", bufs=1) as wrk,
            tc.tile_pool(name="psq", bufs=1, space="PSUM") as psq,
            tc.tile_pool(name="psu", bufs=1, space="PSUM") as psu,
            tc.tile_pool(name="pss", bufs=2, space="PSUM") as pss,
        ):
            # ---- activation LUT preload ----
            scr = wrk.tile([1, 2], F32)
            nc.vector.memset(scr[:], 0.0)
            scr2 = wrk.tile([1, 2], F32)
            nc.scalar.activation(scr2[:, 0:1], scr[:, 0:1], AF.Gelu_apprx_tanh)

            # ---- DMA kicks ----
            # z1's inputs (xt + both w1s halves) head the rings so the
            # z1->gelu chain finishes while dW1 still streams; wd2 rides
            # mid-sync-ring just before the g-terms need it
            # sync:   xt, w1s[0:48], dwa00, wd2, dwa01, dwa1*
            # scalar: xtf8, w1s[48:96], dwb chunks
            # gpsimd: consts in, fo out
            xt_sb = cst.tile([128, KD * B], BF16)
            nc.sync.dma_start(xt_sb[:], XT.ap())
            w1s_sb = cst.tile([128, KD * FSH], BF16)
            nc.sync.dma_start(w1s_sb[:, 0:W1KA * FSH],
                              W1S.ap()[:, 0:W1KA * FSH])
            xtf8_sb = cst.tile([128, KD * B], FP8)
            nc.scalar.dma_start(xtf8_sb[:], XTF8.ap())
            nc.scalar.dma_start(w1s_sb[:, W1KA * FSH:KD * FSH],
                                W1S.ap()[:, W1KA * FSH:KD * FSH])

            def mk_chunk(dram, which, i):
                q, k0, k1 = CHUNKS[i]
                t_ = dwc.tile([128, (k1 - k0) * PW], FP8, name="dwt",
                              tag=f"dw{which}{i}")
                src = dram.ap()[:, (q * KD + k0) * PW:(q * KD + k1) * PW]
                return t_, src

            dwa = []
            wd2_sb = None
            for i in range(len(CHUNKS)):
                t_, src = mk_chunk(DW1A, "a", i)
                nc.sync.dma_start(t_[:], src)
                dwa.append(t_)
                if i == 0:
                    wd2_sb = cst.tile([FSH, (T + 1) * F], BF16)
                    nc.sync.dma_start(wd2_sb[:], WD2.ap())
            # the last two scalar dma_starts are pinned past the gelu
            # activations' sim time so the activations are not stuck
            # behind their semaphore-rotation waits on the scalar engine
            dwb = []
            latepins = {3: 0.032, 4: 0.034}
            for i in range(len(CHUNKS)):
                t_, src = mk_chunk(DW1B, "b", i)
                with tc.tile_wait_until(latepins.get(i, 0),
                                        enable=(i in latepins)):
                    nc.scalar.dma_start(t_[:], src)
                dwb.append(t_)
            cons_sb = cst.tile([128, 35], F32)
            nc.gpsimd.dma_start(cons_sb[:], CONS.ap())
            db1r_sb = cst.tile([B, F], F32)
            nc.gpsimd.dma_start(db1r_sb[:], DB1R.ap())

            eye = cons_sb[0:32, 0:32]
            b1c = cons_sb[0:FSH, 32:33]
            b1p = cons_sb[0:FSH, 33:34]
            b1m = cons_sb[0:FSH, 34:35]
            w2_sb = wd2_sb[:, 0:F]
            xtf8_3 = xtf8_sb[:].rearrange("p (k b) -> p k b", b=B)

            # ---- z1 = X @ W1[:, fc]; two k-parity chains in separate
            # PSUM banks so consecutive matmuls pipeline (same-bank
            # accumulation halves the PE matmul rate) ----
            z1psE = pss.tile([B, FSH], F32, name="sp", tag="sp",
                             padded_shape=[128, 512])
            z1psO = pss.tile([B, FSH], F32, name="sp", tag="sp",
                             padded_shape=[128, 512])
            kseq = list(range(W1KA, KD)) + list(range(0, W1KA))
            for i, k in enumerate(kseq):
                zp = z1psE if i % 2 == 0 else z1psO
                nc.tensor.matmul(zp[:], xt_sb[:, k * B:(k + 1) * B],
                                 w1s_sb[:, k * FSH:(k + 1) * FSH],
                                 start=(i < 2), stop=(i >= KD - 2),
                                 skip_group_check=True)

            z1pad0 = wrk.tile([32, FSH], F32)
            nc.vector.tensor_copy(z1pad0[0:B, :], z1psE[:])
            z1pad = wrk.tile([32, FSH], F32)
            nc.vector.tensor_add(z1pad[0:B, :], z1psO[:], z1pad0[0:B, :])
            z1tp = pss.tile([FSH, 32], F32, name="sp", tag="sp",
                            padded_shape=[128, 512])
            nc.tensor.matmul(z1tp[:], z1pad[:], eye, is_transpose=True,
                             skip_group_check=True)
            z1t = wrk.tile([FSH, B], F32)
            nc.vector.tensor_copy(z1t[:], z1tp[:, 0:B])

            gT = wrk.tile([FSH, B], BF16)
            nc.scalar.activation(gT[:], z1t[:], AF.Gelu_apprx_tanh, bias=b1c)
            gpp = wrk.tile([FSH, B], F32)
            nc.scalar.activation(gpp[:], z1t[:], AF.Gelu_apprx_tanh, bias=b1p)
            gpm = wrk.tile([FSH, B], F32)
            nc.scalar.activation(gpm[:], z1t[:], AF.Gelu_apprx_tanh, bias=b1m)
            gpdr = wrk.tile([FSH, B], F32)
            nc.vector.tensor_sub(gpdr[:], gpp[:], gpm[:])
            gpd = wrk.tile([FSH, B], F32)
            nc.vector.tensor_scalar_mul(gpd[:], gpdr[:], GPD_S)

            gpe = wrk.tile([FSH, 32], BF16)
            nc.vector.memset(gpe[:], 0.0)
            gpo = wrk.tile([FSH, 32], BF16)
            nc.vector.memset(gpo[:], 0.0)
            nc.vector.tensor_copy(gpe[:, 0:16], gT[:])
            nc.vector.tensor_copy(gpo[:, 16:32], gT[:])

            # ---- feats partial ----
            fps5 = pss.tile([B, 512], F32, name="sp", tag="sp",
                            padded_shape=[128, 512])
            nc.tensor.matmul(fps5[:], gT[:], w2_sb[:, 0:512],
                             start=True, stop=True, skip_group_check=True)
            fps2 = pss.tile([B, 256], F32, name="sp", tag="sp",
                            padded_shape=[128, 512])
            nc.tensor.matmul(fps2[:], gT[:], w2_sb[:, 512:F],
                             start=True, stop=True, skip_group_check=True)
            fo_sb = wrk.tile([B, F], F32)
            nc.vector.tensor_copy(fo_sb[:, 0:512], fps5[:])
            nc.vector.tensor_copy(fo_sb[:, 512:F], fps2[:])
            nc.gpsimd.dma_start(FO.ap(), fo_sb[:])

            # ---- P accumulation: pair group gq owns rows 32gq:32gq+32 ----
            P5 = psu.tile([128, 512], F32, name="p5")
            P2 = psu.tile([128, 256], F32, name="p2",
                          padded_shape=[128, 512])
            vps = wrk.tile([FSH, 128], BF16)
            po_sb = wrk.tile([128, F], BF16)

            def mm_gterm(t):
                j = t // 2
                st = gpe if t % 2 == 0 else gpo
                mv = wd2_sb[:, (1 + t) * F:(1 + t) * F + 512]
                nc.tensor.matmul(P5[32 * j:32 * j + 32, :], st[:], mv,
                                 start=(t % 2 == 0), stop=False,
                                 tile_position=(0, 32 * j),
                                 skip_group_check=True)
                mv = wd2_sb[:, (1 + t) * F + 512:(2 + t) * F]
                nc.tensor.matmul(P2[32 * j:32 * j + 32, :], st[:], mv,
                                 start=(t % 2 == 0), stop=False,
                                 tile_position=(0, 32 * j),
                                 skip_group_check=True)

            for t in range(T):
                mm_gterm(t)

            # ---- U accumulators: each pair uses two k-parity chains in
            # SEPARATE PSUM banks (same-bank back-to-back accumulation
            # halves the PE rate); 8 chain tiles rotate through 4 banks,
            # so a pair's banks are recycled after its drain reads them.
            # Creation order matches streaming order (A0, B0, A1, B1).
            upt = {}
            for gq in (0, 2, 1, 3):   # stream order: A-pair0, B-pair0, ...
                for cg in range(2):
                    upt[(gq, cg)] = psq.tile([B, PW], F32,
                                             name=f"u{gq}{cg}", tag="u",
                                             bufs=4,
                                             padded_shape=[128, 512])

            def chunk_u(gq, t_, k0, k1):
                ch3 = t_[:].rearrange("p (k f) -> p k f", f=PW)
                for lp in range((k1 - k0) // 2):
                    kp = k0 // 2 + lp
                    cg = kp % 2
                    nc.tensor.matmul(upt[(gq, cg)][:],
                                     xtf8_3[:, 2 * kp:2 * kp + 2, :],
                                     ch3[:, 2 * lp:2 * lp + 2, :],
                                     start=(kp < 2),
                                     stop=(kp >= KD // 2 - 2),
                                     perf_mode=PM.DoubleRow,
                                     skip_group_check=True)

            def drain_pair(gq):
                for tt in range(2):
                    t = 2 * gq + tt
                    tsum = wrk.tile([B, FSH], F32, name="ts", tag="ts",
                                    bufs=2)
                    nc.vector.tensor_add(
                        tsum[:],
                        upt[(gq, 0)][:, tt * FSH:(tt + 1) * FSH],
                        db1r_sb[:, t * FSH:(t + 1) * FSH])
                    tzpad = wrk.tile([32, FSH], F32, name="tz", tag="tz",
                                     bufs=2)
                    nc.vector.tensor_add(
                        tzpad[0:B, :],
                        upt[(gq, 1)][:, tt * FSH:(tt + 1) * FSH],
                        tsum[:])
                    tztp = pss.tile([FSH, 32], F32, name="sp", tag="sp",
                                    padded_shape=[128, 512])
                    nc.tensor.matmul(tztp[:], tzpad[:], eye,
                                     is_transpose=True,
                                     skip_group_check=True)
                    nc.vector.tensor_mul(vps[:, t * B:(t + 1) * B],
                                         tztp[:, 0:B], gpd[:])
                ro = 32 * gq
                nc.tensor.matmul(P5[ro:ro + 32, :], vps[:, ro:ro + 32],
                                 w2_sb[:, 0:512],
                                 start=False, stop=True,
                                 tile_position=(0, ro),
                                 skip_group_check=True)
                nc.tensor.matmul(P2[ro:ro + 32, :], vps[:, ro:ro + 32],
                                 w2_sb[:, 512:F],
                                 start=False, stop=True,
                                 tile_position=(0, ro),
                                 skip_group_check=True)
                nc.scalar.activation(po_sb[ro:ro + 32, 0:512],
                                     P5[ro:ro + 32, :], AF.Copy)
                nc.vector.tensor_copy(po_sb[ro:ro + 32, 512:F],
                                      P2[ro:ro + 32, :])
                eng = nc.sync if gq < 2 else nc.scalar
                eng.dma_start(PO.ap()[ro:ro + 32, :], po_sb[ro:ro + 32, :])

            # ---- pinned emission in real-arrival order ----
            def piece(ring, i):
                q, k0, k1 = CHUNKS[i]
                gq = q + (0 if ring == "a" else 2)
                t_ = (dwa if ring == "a" else dwb)[i]
                return gq, t_, k0, k1

            sched = [
                ("c", "b", 0, 0.010), ("c", "a", 0, 0.011),
                ("c", "b", 1, 0.012), ("c", "a", 1, 0.013),
                ("d", "b", None, 0.034), ("d", "a", None, 0.0345),
                ("c", "b", 2, 0.035), ("c", "a", 2, 0.036),
                ("c", "b", 3, 0.037), ("c", "b", 4, 0.0375),
                ("c", "a", 3, 0.038), ("c", "a", 4, 0.0385),
                ("d", "b", None, 0.040), ("d", "a", None, 0.044),
            ]
            drained = {"a": 0, "b": 0}
            for kind, ring, i, pin in sched:
                with tc.tile_wait_until(pin):
                    if kind == "c":
                        chunk_u(*piece(ring, i))
                    else:
                        gq = drained[ring] + (0 if ring == "a" else 2)
                        drain_pair(gq)
                        drained[ring] += 1

    nc.compile()
    return nc


def _get_nc():
    if "nc" not in _CACHE:
        _CACHE["nc"] = build()
    return _CACHE["nc"]


def _prep_in_maps(x, W1, b1, W2, b2, mW1, mb1, mW2, mb2, dW1, db1, dW2, db2):
    f32 = np.float32
    bf16 = ml_dtypes.bfloat16
    fp8 = ml_dtypes.float8_e4m3
    X = np.ascontiguousarray(np.asarray(x, f32).reshape(B, D))
    XT = np.ascontiguousarray(X.T)
    XTl = XT.reshape(KD, 128, B).transpose(1, 0, 2).reshape(128, KD * B)
    XTb = np.ascontiguousarray(XTl).astype(bf16)
    XTf8 = np.ascontiguousarray(XTl * FP8_XS).astype(fp8)
    W1 = np.asarray(W1, f32)
    W2 = np.asarray(W2, f32)
    b1 = np.asarray(b1, f32)
    dW1 = np.asarray(dW1, f32)
    db1 = np.asarray(db1, f32)
    dW2 = np.asarray(dW2, f32)

    def pack_pairs(dw_half, fc):
        blocks = []
        for q in range(2):
            arr = (dw_half[2 * q:2 * q + 2, :, fc] * FP8_SCALE)
            arr = arr.transpose(1, 0, 2).reshape(KD, 128, PW)
            blocks.append(arr.transpose(1, 0, 2).reshape(128, KD * PW))
        return np.ascontiguousarray(np.concatenate(blocks, 1)).astype(fp8)

    in_maps = []
    for c in range(NCORES):
        fc = slice(c * FSH, (c + 1) * FSH)
        w1s = np.ascontiguousarray(
            W1[:, fc].reshape(KD, 128, FSH).transpose(1, 0, 2)
            .reshape(128, KD * FSH)).astype(bf16)
        wd2 = np.concatenate(
            [W2[fc, :][:, None, :],
             dW2[:, fc, :].transpose(1, 0, 2)], axis=1).reshape(FSH,
                                                               (T + 1) * F)
        cons = np.zeros((128, 35), f32)
        cons[0:32, 0:32] = np.eye(32, dtype=f32)
        cons[0:FSH, 32] = b1[fc]
        cons[0:FSH, 33] = b1[fc] + EPS
        cons[0:FSH, 34] = b1[fc] - EPS
        db1r = np.ascontiguousarray(np.broadcast_to(
            (FP8_SCALE * FP8_XS * db1[:, fc]).reshape(T * FSH), (B, F)))
        in_maps.append({
            "xt": XTb,
            "xtf8": XTf8,
            "w1s": w1s,
            "dw1a": pack_pairs(dW1[0:4], fc),
            "dw1b": pack_pairs(dW1[4:8], fc),
            "wd2": np.ascontiguousarray(wd2).astype(bf16),
            "cons": cons,
            "db1r": db1r.astype(f32),
        })
    return in_maps


def run(inputs, trace=False, trace_cores=None, tmpdir=None):
    nc = _get_nc()
    in_maps = _prep_in_maps(**inputs)
    res = bass_utils.run_bass_kernel_spmd(
        nc, in_maps, core_ids=list(range(NCORES)), trace=trace,
        trace_cores=trace_cores, tmpdir=tmpdir)

    f64 = np.float64
    b2 = np.asarray(inputs["b2"], f64)
    mW1 = np.asarray(inputs["mW1"], f64)
    mb1 = np.asarray(inputs["mb1"], f64)
    mW2 = np.asarray(inputs["mW2"], f64)
    mb2 = np.asarray(inputs["mb2"], f64)
    db2 = np.asarray(inputs["db2"], f64)

    feats = np.zeros((B, F), f64)
    P = np.zeros((128, F), f64)
    for c in range(NCORES):
        feats += res.results[c]["fo"].astype(f64)
        P += res.results[c]["po"].astype(f64)
    feats += b2[None, :]
    h = np.maximum(feats @ mW1.T + mb1, 0.0)
    coefs = h @ mW2.T + mb2                     # [B, T]
    out = feats + coefs @ db2
    for t in range(T):
        out += coefs[:, t:t + 1] * P[t * B:(t + 1) * B]
    return out.astype(np.float32), res


def kernel(**inputs):
    out, _ = run(inputs, trace=False)
    return out
